# revision 7
# baseline (speedup 1.0000x reference)
# BitLinear (eval path) Trainium2 kernel: ternary weight quant + int8 activation
# quant + dense matmul, tensor-parallel over 8 NeuronCores.
#
# Math (per reference):
#   w_scale[o] = max(mean_k |W[o,k]|, EPS)
#   w_quant    = clip(round(W / w_scale), -1, 1)            (ternary)
#   x_scale[t] = max(max_k |x[t,k]| / 127, EPS)
#   x_quant    = round(x / x_scale)                          (int8 range)
#   out[t,o]   = (sum_k x_quant[t,k] * w_quant[o,k]) * x_scale[t] * w_scale[o] + bias[o]
#
# The integer sum is computed exactly on the PE: w_quant is exact in fp8e4,
# x_quant (|v| <= 127) is exact in bf16, products/partials are exact in the
# fp32 PSUM accumulator (max |sum| <= 127*4096 < 2^24).
#
# Sharding: 4 token groups x 2 out-feature groups = 8 cores. Host passes
# transposed (K-major) layouts so both matmul operands stream with K on
# partitions. Per core the matmul is output-major: the fp8 weight slice is
# the PE's stationary operand and the bf16 activations stream 512 tokens at
# a time, so PSUM lands as [o, t] and the host transposes the gathered
# output once. W-scale reduction runs on DVE+GpSimd (no PE involvement) and
# the quantize multiplies run on GpSimd so the vector engine stays below the
# PE's ~460us of matmul work; W/X quantization is interleaved between
# matmul passes instead of running as a serial prologue.
import numpy as np

import concourse.bacc as bacc
import concourse.bass as bass
import concourse.bass_isa as bass_isa
import concourse.tile as tile
from concourse import mybir
from concourse.bass_utils import run_bass_kernel_spmd

F32 = mybir.dt.float32
BF16 = mybir.dt.bfloat16
FP8 = mybir.dt.float8e4

EPS = 1e-5
MAGIC = 12582912.0  # 1.5 * 2^23: (x + MAGIC) - MAGIC == rint(x) for |x| < 2^22

# Full-problem shapes (hardcoded per contract).
B, S, I, O = 4, 2048, 4096, 4096
T_FULL = B * S  # 8192 tokens
TSPLIT, OSPLIT = 4, 2  # token groups x out-feature groups = 8 cores
N_CORES = TSPLIT * OSPLIT

A = mybir.AluOpType


def build_nc(K=I, TO=O // OSPLIT, TT=T_FULL // TSPLIT):
    """Per-core Bass program; every core runs the same program on its shard:
    xT [K, TT], wT [K, TO], bias [TO] -> out [TO, TT] (o-major)."""
    KT = K // 128        # 32 k subtiles
    QT = 512             # tokens per quarter-section
    NQ = TT // QT        # 4 quarter-sections
    GT = 128             # tokens per amax group
    NGQ = QT // GT       # 4 groups per quarter
    OB = 256             # W block o-columns
    NOB = TO // OB       # 8 W blocks
    KH = KT // 2         # half-kt staging (16)
    NOC = TO // 128      # 16 output-column passes

    nc = bacc.Bacc("TRN2", target_bir_lowering=False, debug=False)
    xT = nc.dram_tensor("xT", [K, TT], F32, kind="ExternalInput").ap()
    wT = nc.dram_tensor("wT", [K, TO], F32, kind="ExternalInput").ap()
    bias_d = nc.dram_tensor("bias", [TO], F32, kind="ExternalInput").ap()
    out_d = nc.dram_tensor("out", [TO, TT], F32, kind="ExternalOutput").ap()

    # K-major DRAM views: [p, kt, cols]
    x_v = xT.rearrange("(kt p) t -> p kt t", p=128)
    w_v = wT.rearrange("(kt p) o -> p kt o", p=128)
    bias_v = bias_d.rearrange("(i p) -> p i", p=128)  # [128, 16] col-major

    with tile.TileContext(nc) as tc:
        with (
            tc.tile_pool(name="stg", bufs=3) as p_stg,    # f32 staging (W halves / x groups)
            tc.tile_pool(name="wq", bufs=1) as p_wq,      # resident fp8 weights
            tc.tile_pool(name="xq", bufs=2) as p_xq,      # resident bf16 activations (quarter)
            tc.tile_pool(name="xs", bufs=2) as p_xs,      # per-quarter amax/scale rows
            tc.tile_pool(name="small", bufs=3) as p_small,  # per-group amax rows
            tc.tile_pool(name="wsml", bufs=2) as p_wsml,  # W-scale small tiles
            tc.tile_pool(name="osb", bufs=3) as p_osb,    # epilogue staging
            tc.tile_pool(name="const", bufs=1) as p_const,
            tc.tile_pool(name="ps_mm", bufs=5, space="PSUM") as ps_mm,
            tc.tile_pool(name="ps_tr", bufs=2, space="PSUM") as ps_tr,
        ):
            ones11 = p_const.tile([1, 1], F32)
            nc.vector.memset(ones11[:], 1.0)
            ws_cols = p_const.tile([128, NOC], F32)   # w_scale, o on partitions
            bias_cols = p_const.tile([128, NOC], F32)  # bias, o on partitions
            nc.sync.dma_start(out=bias_cols[:], in_=bias_v)

            # Resident quantized weights: one tile per 256-col block so matmul
            # passes only depend on the blocks they read.
            wq_blocks = [
                p_wq.tile([128, KT, OB], FP8, name=f"wqb_{ob}") for ob in range(NOB)
            ]

            # ---------- x quarter: 4 groups of 128 tokens ----------
            def make_x_quarter(q):
                """Allocate quarter tiles and return per-group work closures."""
                xq_t = p_xq.tile([128, KT, QT], BF16, tag="xq")
                xs_t = p_xs.tile([128, QT], F32, tag="xs")
                rxs_t = p_xs.tile([128, QT], F32, tag="rxs")
                amq_t = p_xs.tile([128, QT], F32, tag="amq")

                def group(g):
                    lo, hi = g * GT, (g + 1) * GT
                    xg = p_stg.tile([128, KT, GT], F32, tag="stg")
                    nc.sync.dma_start(
                        out=xg[:], in_=x_v[:, :, q * QT + lo : q * QT + hi]
                    )
                    # amax over kt (transposed view), |.| applied
                    am = p_small.tile([128, GT], F32, tag="am")
                    nc.vector.tensor_reduce(
                        out=am[:],
                        in_=xg[:].rearrange("p kt t -> p t kt"),
                        axis=mybir.AxisListType.X,
                        op=A.max,
                        apply_absolute_value=True,
                    )
                    # partition-dim max, broadcast to all lanes
                    nc.gpsimd.partition_all_reduce(
                        amq_t[:, lo:hi], am[:], 128, bass_isa.ReduceOp.absmax
                    )
                    nc.vector.tensor_scalar(
                        out=xs_t[:, lo:hi], in0=amq_t[:, lo:hi],
                        scalar1=1.0 / 127.0, scalar2=EPS, op0=A.mult, op1=A.max,
                    )
                    nc.vector.reciprocal(rxs_t[:, lo:hi], xs_t[:, lo:hi])
                    # x *= 1/xs (in place, on gpsimd), then round -> bf16
                    rxs_kt = bass.AP(
                        tensor=rxs_t.tensor,
                        offset=rxs_t[:, lo:hi].offset,
                        ap=[rxs_t.ap[0], [0, KT], [1, GT]],
                    )
                    nc.gpsimd.tensor_tensor(out=xg[:], in0=xg[:], in1=rxs_kt, op=A.mult)
                    nc.vector.tensor_scalar(
                        out=xq_t[:, :, lo:hi], in0=xg[:],
                        scalar1=MAGIC, scalar2=MAGIC, op0=A.add, op1=A.subtract,
                    )

                return xq_t, xs_t, group

            # ---------------- W block: scales + ternary quantization ----------
            def w_block(ob):
                halves = []
                parts = []
                for h in range(2):
                    wgf = p_stg.tile([128, KH, OB], F32, tag="stg")
                    nc.sync.dma_start(
                        out=wgf[:],
                        in_=w_v[:, h * KH : (h + 1) * KH, ob * OB : (ob + 1) * OB],
                    )
                    part = p_wsml.tile([128, OB], F32, tag=f"part{h}")
                    nc.vector.tensor_reduce(
                        out=part[:],
                        in_=wgf[:].rearrange("p kt o -> p o kt"),
                        axis=mybir.AxisListType.X,
                        op=A.add,
                        apply_absolute_value=True,
                    )
                    halves.append(wgf)
                    parts.append(part)
                wsum = p_wsml.tile([128, OB], F32, tag="wsum")
                nc.vector.tensor_tensor(
                    out=wsum[:], in0=parts[0][:], in1=parts[1][:], op=A.add
                )
                wsum_bc = p_wsml.tile([128, OB], F32, tag="wsumbc")
                nc.gpsimd.partition_all_reduce(
                    wsum_bc[:], wsum[:], 128, bass_isa.ReduceOp.add
                )
                # w_scale = max(sum/K, EPS) broadcast on all partitions
                ws_bc = p_wsml.tile([128, OB], F32, tag="wsbc")
                nc.vector.tensor_scalar(
                    out=ws_bc[:], in0=wsum_bc[:], scalar1=1.0 / K, scalar2=EPS,
                    op0=A.mult, op1=A.max,
                )
                rws_bc = p_wsml.tile([128, OB], F32, tag="rwsbc")
                nc.vector.reciprocal(rws_bc[:], ws_bc[:])
                # transpose the two 128-wide ws row slices into ws_cols
                for j in range(OB // 128):
                    ptr = ps_tr.tile([128, 1], F32, tag="tr")
                    nc.tensor.transpose(
                        ptr[:], ws_bc[0:1, j * 128 : (j + 1) * 128], ones11[0:1, 0:1]
                    )
                    oc = ob * (OB // 128) + j
                    nc.scalar.copy(ws_cols[:, oc : oc + 1], ptr[:])
                # quantize both halves: w *= 1/ws (gpsimd), round+clip -> fp8
                for h in range(2):
                    wgf = halves[h]
                    rws_kt = bass.AP(
                        tensor=rws_bc.tensor,
                        offset=rws_bc.offset,
                        ap=[rws_bc.ap[0], [0, KH], rws_bc.ap[1]],
                    )
                    nc.gpsimd.tensor_tensor(
                        out=wgf[:], in0=wgf[:], in1=rws_kt, op=A.mult
                    )
                    # (v + M) min (M+1): exact rint then upper clip
                    nc.vector.tensor_scalar(
                        out=wgf[:], in0=wgf[:], scalar1=MAGIC, scalar2=MAGIC + 1.0,
                        op0=A.add, op1=A.min,
                    )
                    # (v max (M-1)) - M: lower clip, remove magic, cast fp8
                    nc.vector.tensor_scalar(
                        out=wq_blocks[ob][:, h * KH : (h + 1) * KH, :],
                        in0=wgf[:], scalar1=MAGIC - 1.0, scalar2=MAGIC,
                        op0=A.max, op1=A.subtract,
                    )

            # ---------------- matmul pass: one (quarter, oc128) pair ----------
            def mm_pass(q, oc, xq_t, xs_t):
                pm = ps_mm.tile([128, QT], F32, tag="mm")
                wqb = wq_blocks[oc // 2]
                osl = (oc % 2) * 128
                for kt in range(KT):
                    nc.tensor.matmul(
                        pm[:],
                        wqb[:, kt, osl : osl + 128],
                        xq_t[:, kt, :],
                        start=(kt == 0),
                        stop=(kt == KT - 1),
                    )
                osb = p_osb.tile([128, QT], F32, tag="osb")
                # (psum * ws[o]) * xs[t]
                nc.vector.scalar_tensor_tensor(
                    out=osb[:], in0=pm[:], scalar=ws_cols[:, oc : oc + 1],
                    in1=xs_t[:], op0=A.mult, op1=A.mult,
                )
                # + bias[o] (scalar engine, per-partition bias)
                nc.scalar.activation(
                    out=osb[:], in_=osb[:],
                    func=mybir.ActivationFunctionType.Identity,
                    bias=bias_cols[:, oc : oc + 1],
                )
                nc.sync.dma_start(
                    out=out_d[oc * 128 : (oc + 1) * 128, q * QT : (q + 1) * QT],
                    in_=osb[:],
                )

            # ---------------- main schedule ----------
            # Fill: quarter 0 activations, then first W blocks.
            xq_t, xs_t, group = make_x_quarter(0)
            for g in range(NGQ):
                group(g)
            w_block(0)
            w_block(1)
            wb_next = 2
            cur = (xq_t, xs_t)
            nxt = None
            for q in range(NQ):
                # pre-allocate next quarter's closures
                if q + 1 < NQ:
                    nxq, nxs, ngroup = make_x_quarter(q + 1)
                for oc in range(NOC):
                    # interleave remaining W blocks early in quarter 0
                    if q == 0 and wb_next < NOB and oc in (1, 2, 3, 4, 5, 6):
                        w_block(wb_next)
                        wb_next += 1
                    # interleave next-quarter activation groups mid-quarter
                    if q + 1 < NQ and oc in (7, 9, 11, 13):
                        ngroup((oc - 7) // 2)
                    mm_pass(q, oc, *cur)
                if q + 1 < NQ:
                    cur = (nxq, nxs)
    nc.compile()
    return nc


_NC_CACHE = {}
LAST_EXEC_NS = None


def _get_nc():
    key = "full"
    if key not in _NC_CACHE:
        _NC_CACHE[key] = build_nc()
    return _NC_CACHE[key]


def _run(x, weight, bias, trace=False):
    global LAST_EXEC_NS
    x = np.asarray(x, dtype=np.float32)
    weight = np.asarray(weight, dtype=np.float32)
    bias = np.asarray(bias, dtype=np.float32)

    xT = np.ascontiguousarray(x.reshape(T_FULL, I).T)  # [I, T]
    wT = np.ascontiguousarray(weight.T)  # [I, O]

    TT = T_FULL // TSPLIT
    TO = O // OSPLIT
    in_maps = []
    for c in range(N_CORES):
        ti, oj = divmod(c, OSPLIT)
        in_maps.append(
            {
                "xT": np.ascontiguousarray(xT[:, ti * TT : (ti + 1) * TT]),
                "wT": np.ascontiguousarray(wT[:, oj * TO : (oj + 1) * TO]),
                "bias": np.ascontiguousarray(bias[oj * TO : (oj + 1) * TO]),
            }
        )

    nc = _get_nc()
    res = run_bass_kernel_spmd(
        nc, in_maps, core_ids=list(range(N_CORES)), trace=trace
    )
    LAST_EXEC_NS = res.exec_time_ns

    out = np.empty((T_FULL, O), dtype=np.float32)
    for c in range(N_CORES):
        ti, oj = divmod(c, OSPLIT)
        out[ti * TT : (ti + 1) * TT, oj * TO : (oj + 1) * TO] = res.results[c][
            "out"
        ].T
    return out.reshape(B, S, O)


def kernel(x, weight, bias):
    return _run(x, weight, bias, trace=False)


def kernel_traced(x, weight, bias):
    _run(x, weight, bias, trace=True)
    return LAST_EXEC_NS


# revision 10
# speedup vs baseline: 1.0842x; 1.0842x over previous
# BitLinear (eval path) Trainium2 kernel: ternary weight quant + int8 activation
# quant + dense matmul, tensor-parallel over 8 NeuronCores.
#
# Math (per reference):
#   w_scale[o] = max(mean_k |W[o,k]|, EPS)
#   w_quant    = clip(round(W / w_scale), -1, 1)            (ternary)
#   x_scale[t] = max(max_k |x[t,k]| / 127, EPS)
#   x_quant    = round(x / x_scale)                          (int8 range)
#   out[t,o]   = (sum_k x_quant[t,k] * w_quant[o,k]) * x_scale[t] * w_scale[o] + bias[o]
#
# Exactness: w_quant exact in fp8e4, x_quant (|v|<=127) exact in bf16,
# partials exact in fp32 PSUM (<= 127*4096 < 2^24).
#
# Sharding: 4 token groups x 2 out-feature groups = 8 cores. Matmuls are
# token-major (bf16 x_quant stationary, fp8 weights moving 512 wide).
# Weights stream from HBM TWICE (pass 1: |w| row sums -> scales; pass 2:
# quantize) so no large raw-W SBUF residency is needed. Activation amax uses
# scalar-engine |.| + a contiguous max tree on DVE (strided reduces cost
# 2 cyc/elem); all DVE quant ops write fresh tiles (in-place loses the 2x
# perf mode); scale multiplies run on GpSimd. Everything is interleaved with
# the matmul passes in token-section order so the PE never waits long.
import numpy as np

import concourse.bacc as bacc
import concourse.bass as bass
import concourse.bass_isa as bass_isa
import concourse.tile as tile
from concourse import mybir
from concourse.bass_utils import run_bass_kernel_spmd

F32 = mybir.dt.float32
BF16 = mybir.dt.bfloat16
FP8 = mybir.dt.float8e4

EPS = 1e-5
MAGIC = 12582912.0  # 1.5 * 2^23: (x + MAGIC) - MAGIC == rint(x) for |x| < 2^22

B, S, I, O = 4, 2048, 4096, 4096
T_FULL = B * S
TSPLIT, OSPLIT = 4, 2
N_CORES = TSPLIT * OSPLIT

A = mybir.AluOpType


def build_nc(K=I, TO=O // OSPLIT, TT=T_FULL // TSPLIT):
    """Per-core program: xT [K, TT], wT [K, TO], bias [TO] -> out [TT, TO]."""
    KT = K // 128      # 32 k subtiles
    KH = 16            # x half-unit kt size
    KQ = 8             # W quarter-unit kt size
    GT = 128           # tokens per group
    NG = TT // GT      # 16 token groups
    OC = 512           # moving width per matmul
    NOC = TO // OC     # 4 o-chunks
    OB = 256           # W quant block o-columns
    NOB = TO // OB     # 8 W blocks

    nc = bacc.Bacc("TRN2", target_bir_lowering=False, debug=False)
    xT = nc.dram_tensor("xT", [K, TT], F32, kind="ExternalInput").ap()
    wT = nc.dram_tensor("wT", [K, TO], F32, kind="ExternalInput").ap()
    bias_d = nc.dram_tensor("bias", [TO], F32, kind="ExternalInput").ap()
    out_d = nc.dram_tensor("out", [TT, TO], F32, kind="ExternalOutput").ap()

    x_v = xT.rearrange("(kt p) t -> p kt t", p=128)
    w_v = wT.rearrange("(kt p) o -> p kt o", p=128)

    with tile.TileContext(nc) as tc:
        with (
            tc.tile_pool(name="ld", bufs=4) as p_ld,      # 8KB dma staging units
            tc.tile_pool(name="ax", bufs=3) as p_ax,      # 8KB abs/mult scratch
            tc.tile_pool(name="th", bufs=2) as p_th,      # 4KB tree scratch
            tc.tile_pool(name="wq", bufs=1) as p_wq,      # resident fp8 weights
            tc.tile_pool(name="xq", bufs=5) as p_xq,      # bf16 token-group tiles
            tc.tile_pool(name="sml", bufs=2) as p_sml,
            tc.tile_pool(name="rws", bufs=3) as p_rws,    # 1/w_scale, pass1->pass2
            tc.tile_pool(name="osb", bufs=2) as p_osb,
            tc.tile_pool(name="const", bufs=1) as p_const,
            tc.tile_pool(name="ps_mm", bufs=5, space="PSUM") as ps_mm,
            tc.tile_pool(name="ps_tr", bufs=2, space="PSUM") as ps_tr,
        ):
            ones11 = p_const.tile([1, 1], F32)
            nc.vector.memset(ones11[:], 1.0)
            xs_cols = p_const.tile([128, NG], F32)    # x_scale, t on partitions
            ws_full = p_const.tile([128, TO], F32)    # w_scale bcast rows
            bias_bc = p_const.tile([128, TO], F32)    # bias bcast rows
            nc.gpsimd.dma_start(
                out=bias_bc[:],
                in_=bass.AP(
                    tensor=bias_d.tensor, offset=bias_d.offset,
                    ap=[[0, 128], [1, TO]],
                ),
            )

            wq_oc = [
                p_wq.tile([128, KT, OC], FP8, name=f"wq_{oc}") for oc in range(NOC)
            ]
            xq_tiles = {}

            # ---------- x group: 2 half-units, amax tree, quantize ----------
            def x_group(tg):
                ts = slice(tg * GT, (tg + 1) * GT)
                xgs, axs = [], []
                for h in range(2):
                    xg = p_ld.tile([128, KH, GT], F32, tag="ld")
                    nc.sync.dma_start(
                        out=xg[:], in_=x_v[:, h * KH : (h + 1) * KH, ts]
                    )
                    xgs.append(xg)
                for h in range(2):
                    ax = p_ax.tile([128, KH, GT], F32, tag="ax")
                    nc.scalar.activation(
                        out=ax[:], in_=xgs[h][:],
                        func=mybir.ActivationFunctionType.Abs,
                    )
                    axs.append(ax)
                for h in range(2):
                    ax = axs[h]
                    th = p_th.tile([128, 8, GT], F32, tag="th")
                    # contiguous max tree over kt: 16->8->4->2->1
                    nc.vector.tensor_tensor(
                        out=th[:, 0:8, :], in0=ax[:, 0:8, :], in1=ax[:, 8:16, :],
                        op=A.max,
                    )
                    nc.vector.tensor_tensor(
                        out=ax[:, 0:4, :], in0=th[:, 0:4, :], in1=th[:, 4:8, :],
                        op=A.max,
                    )
                    nc.vector.tensor_tensor(
                        out=th[:, 0:2, :], in0=ax[:, 0:2, :], in1=ax[:, 2:4, :],
                        op=A.max,
                    )
                    nc.vector.tensor_tensor(
                        out=ax[:, 0:1, :], in0=th[:, 0:1, :], in1=th[:, 1:2, :],
                        op=A.max,
                    )
                am = p_sml.tile([128, GT], F32, tag="am")
                nc.vector.tensor_tensor(
                    out=am[:], in0=axs[0][:, 0, :], in1=axs[1][:, 0, :], op=A.max
                )
                am_bc = p_sml.tile([128, GT], F32, tag="ambc")
                nc.gpsimd.partition_all_reduce(
                    am_bc[:], am[:], 128, bass_isa.ReduceOp.absmax
                )
                xs_bc = p_sml.tile([128, GT], F32, tag="xsbc")
                nc.vector.tensor_scalar(
                    out=xs_bc[:], in0=am_bc[:], scalar1=1.0 / 127.0, scalar2=EPS,
                    op0=A.mult, op1=A.max,
                )
                rxs_bc = p_sml.tile([128, GT], F32, tag="rxsbc")
                nc.vector.reciprocal(rxs_bc[:], xs_bc[:])
                ptr = ps_tr.tile([128, 1], F32, tag="tr")
                nc.tensor.transpose(ptr[:], xs_bc[0:1, :], ones11[0:1, 0:1])
                nc.scalar.copy(xs_cols[:, tg : tg + 1], ptr[:])
                rxs_kt = bass.AP(
                    tensor=rxs_bc.tensor, offset=rxs_bc.offset,
                    ap=[rxs_bc.ap[0], [0, KH], rxs_bc.ap[1]],
                )
                xq_t = p_xq.tile([128, KT, GT], BF16, tag="xq")
                for h in range(2):
                    # xn = x * (1/xs) into the abs tile (free after the tree)
                    nc.gpsimd.tensor_tensor(
                        out=axs[h][:], in0=xgs[h][:], in1=rxs_kt, op=A.mult
                    )
                for h in range(2):
                    nc.vector.tensor_scalar(
                        out=xq_t[:, h * KH : (h + 1) * KH, :], in0=axs[h][:],
                        scalar1=MAGIC, scalar2=MAGIC, op0=A.add, op1=A.subtract,
                    )
                xq_tiles[tg] = xq_t

            # ---------- W pass 1: stream block, |w| row sums -> scales ----------
            def w_scales(ob):
                osl = slice(ob * OB, (ob + 1) * OB)
                run = None
                for qx in range(4):
                    wgf = p_ld.tile([128, KQ, OB], F32, tag="ld")
                    nc.sync.dma_start(
                        out=wgf[:], in_=w_v[:, qx * KQ : (qx + 1) * KQ, osl]
                    )
                    part = p_sml.tile([128, OB], F32, tag="part")
                    nc.vector.tensor_reduce(
                        out=part[:],
                        in_=wgf[:].rearrange("p kt o -> p o kt"),
                        axis=mybir.AxisListType.X,
                        op=A.add,
                        apply_absolute_value=True,
                    )
                    if run is None:
                        run = part
                    else:
                        nrun = p_sml.tile([128, OB], F32, tag="run")
                        nc.vector.tensor_tensor(
                            out=nrun[:], in0=run[:], in1=part[:], op=A.add
                        )
                        run = nrun
                wsum_bc = p_sml.tile([128, OB], F32, tag="wsumbc")
                nc.gpsimd.partition_all_reduce(
                    wsum_bc[:], run[:], 128, bass_isa.ReduceOp.add
                )
                nc.vector.tensor_scalar(
                    out=ws_full[:, osl], in0=wsum_bc[:],
                    scalar1=1.0 / K, scalar2=EPS, op0=A.mult, op1=A.max,
                )
                rws_bc = p_rws.tile([128, OB], F32, tag="rws")
                nc.vector.reciprocal(rws_bc[:], ws_full[:, osl])
                return rws_bc

            # ---------- W pass 2: re-stream block, quantize -> fp8 ----------
            def w_quant(ob, rws_bc):
                osl = slice(ob * OB, (ob + 1) * OB)
                rws_kt = bass.AP(
                    tensor=rws_bc.tensor, offset=rws_bc.offset,
                    ap=[rws_bc.ap[0], [0, KQ], rws_bc.ap[1]],
                )
                oc, oco = ob // 2, (ob % 2) * OB
                for qx in range(4):
                    wgf = p_ld.tile([128, KQ, OB], F32, tag="ld")
                    nc.sync.dma_start(
                        out=wgf[:], in_=w_v[:, qx * KQ : (qx + 1) * KQ, osl]
                    )
                    mq = p_ax.tile([128, KQ, OB], F32, tag="ax")
                    nc.gpsimd.tensor_tensor(
                        out=mq[:], in0=wgf[:], in1=rws_kt, op=A.mult
                    )
                    # (v + M) min (M+1): exact rint + upper clip (reuse wgf)
                    nc.vector.tensor_scalar(
                        out=wgf[:], in0=mq[:], scalar1=MAGIC, scalar2=MAGIC + 1.0,
                        op0=A.add, op1=A.min,
                    )
                    # (v max (M-1)) - M: lower clip, remove magic, cast fp8
                    nc.vector.tensor_scalar(
                        out=wq_oc[oc][:, qx * KQ : (qx + 1) * KQ, oco : oco + OB],
                        in0=wgf[:], scalar1=MAGIC - 1.0, scalar2=MAGIC,
                        op0=A.max, op1=A.subtract,
                    )

            def w_block_pair(ob0):
                r0 = w_scales(ob0)
                r1 = w_scales(ob0 + 1)
                w_quant(ob0, r0)
                w_quant(ob0 + 1, r1)

            # ---------- matmul pass ----------
            def mm_pass(tg, oc):
                xq_t = xq_tiles[tg]
                pm = ps_mm.tile([128, OC], F32, tag="mm")
                for kt in range(KT):
                    nc.tensor.matmul(
                        pm[:],
                        xq_t[:, kt, :],
                        wq_oc[oc][:, kt, :],
                        start=(kt == 0),
                        stop=(kt == KT - 1),
                    )
                osb = p_osb.tile([128, OC], F32, tag="osb")
                nc.vector.scalar_tensor_tensor(
                    out=osb[:], in0=pm[:], scalar=xs_cols[:, tg : tg + 1],
                    in1=ws_full[:, oc * OC : (oc + 1) * OC], op0=A.mult, op1=A.mult,
                )
                nc.gpsimd.tensor_tensor(
                    out=osb[:], in0=osb[:],
                    in1=bias_bc[:, oc * OC : (oc + 1) * OC], op=A.add,
                )
                nc.sync.dma_start(
                    out=out_d[tg * GT : (tg + 1) * GT, oc * OC : (oc + 1) * OC],
                    in_=osb[:],
                )

            # ---------- main schedule ----------
            # Fill: first token section + W blocks 0,1 (for oc 0).
            x_group(0)
            x_group(1)
            w_block_pair(0)
            x_group(2)
            x_group(3)
            wb_next = 2
            for sec in range(4):
                for oc in range(NOC):
                    for tg in range(sec * 4, sec * 4 + 4):
                        mm_pass(tg, oc)
                    # W pair for oc+1 issued one slot ahead of its readers
                    if sec == 0 and wb_next < NOB:
                        w_block_pair(wb_next)
                        wb_next += 2
                    # prefetch next section's token groups (after the mms so
                    # parked rounds never sit ahead of this slot's epilogues)
                    if sec < 3 and oc == 2:
                        for g in range(sec * 4 + 4, sec * 4 + 8):
                            x_group(g)
    nc.compile()
    return nc


_NC_CACHE = {}
LAST_EXEC_NS = None


def _get_nc():
    if "full" not in _NC_CACHE:
        _NC_CACHE["full"] = build_nc()
    return _NC_CACHE["full"]


def _run(x, weight, bias, trace=False):
    global LAST_EXEC_NS
    x = np.asarray(x, dtype=np.float32)
    weight = np.asarray(weight, dtype=np.float32)
    bias = np.asarray(bias, dtype=np.float32)

    xT = np.ascontiguousarray(x.reshape(T_FULL, I).T)  # [I, T]
    wT = np.ascontiguousarray(weight.T)  # [I, O]

    TT = T_FULL // TSPLIT
    TO = O // OSPLIT
    in_maps = []
    for c in range(N_CORES):
        ti, oj = divmod(c, OSPLIT)
        in_maps.append(
            {
                "xT": np.ascontiguousarray(xT[:, ti * TT : (ti + 1) * TT]),
                "wT": np.ascontiguousarray(wT[:, oj * TO : (oj + 1) * TO]),
                "bias": np.ascontiguousarray(bias[oj * TO : (oj + 1) * TO]),
            }
        )

    nc = _get_nc()
    res = run_bass_kernel_spmd(
        nc, in_maps, core_ids=list(range(N_CORES)), trace=trace
    )
    LAST_EXEC_NS = res.exec_time_ns

    out = np.empty((T_FULL, O), dtype=np.float32)
    for c in range(N_CORES):
        ti, oj = divmod(c, OSPLIT)
        out[ti * TT : (ti + 1) * TT, oj * TO : (oj + 1) * TO] = res.results[c]["out"]
    return out.reshape(B, S, O)


def kernel(x, weight, bias):
    return _run(x, weight, bias, trace=False)


def kernel_traced(x, weight, bias):
    _run(x, weight, bias, trace=True)
    return LAST_EXEC_NS


# revision 20
# speedup vs baseline: 1.0933x; 1.0084x over previous
# BitLinear (eval path) Trainium2 kernel: ternary weight quant + int8 activation
# quant + dense matmul, tensor-parallel over 8 NeuronCores.
#
# Math (per reference):
#   w_scale[o] = max(mean_k |W[o,k]|, EPS)
#   w_quant    = clip(round(W / w_scale), -1, 1)            (ternary)
#   x_scale[t] = max(max_k |x[t,k]| / 127, EPS)
#   x_quant    = round(x / x_scale)                          (int8 range)
#   out[t,o]   = (sum_k x_quant[t,k] * w_quant[o,k]) * x_scale[t] * w_scale[o] + bias[o]
#
# Exactness: w_quant exact in fp8e4, x_quant (|v|<=127) exact in bf16,
# partials exact in fp32 PSUM (<= 127*4096 < 2^24).
#
# Sharding: 4 token groups x 2 out-feature groups = 8 cores. Matmuls are
# token-major (bf16 x_quant stationary, fp8 weights moving 512 wide).
# Weights stream from HBM TWICE (pass 1: |w| row sums -> scales; pass 2:
# quantize) so no large raw-W SBUF residency is needed. Activation amax uses
# scalar-engine |.| + a contiguous max tree on DVE (strided reduces cost
# 2 cyc/elem); all DVE quant ops write fresh tiles (in-place loses the 2x
# perf mode); scale multiplies run on GpSimd. Everything is interleaved with
# the matmul passes in token-section order so the PE never waits long.
import numpy as np

import concourse.bacc as bacc
import concourse.bass as bass
import concourse.bass_isa as bass_isa
import concourse.tile as tile
from concourse import mybir
from concourse.bass_utils import run_bass_kernel_spmd

F32 = mybir.dt.float32
BF16 = mybir.dt.bfloat16
FP8 = mybir.dt.float8e4

EPS = 1e-5
MAGIC = 12582912.0  # 1.5 * 2^23: (x + MAGIC) - MAGIC == rint(x) for |x| < 2^22

B, S, I, O = 4, 2048, 4096, 4096
T_FULL = B * S
TSPLIT, OSPLIT = 4, 2
N_CORES = TSPLIT * OSPLIT

A = mybir.AluOpType


def build_nc(K=I, TO=O // OSPLIT, TT=T_FULL // TSPLIT):
    """Per-core program: xT [K, TT], wT [K, TO], bias [TO] -> out [TT, TO]."""
    KT = K // 128      # 32 k subtiles
    KH = 16            # x half-unit kt size
    KQ = 8             # W quarter-unit kt size
    GT = 128           # tokens per group
    NG = TT // GT      # 16 token groups
    OC = 512           # moving width per matmul
    NOC = TO // OC     # 4 o-chunks
    OB = 256           # W quant block o-columns
    NOB = TO // OB     # 8 W blocks

    nc = bacc.Bacc("TRN2", target_bir_lowering=False, debug=False)
    xT = nc.dram_tensor("xT", [K, TT], F32, kind="ExternalInput").ap()
    wT = nc.dram_tensor("wT", [K, TO], F32, kind="ExternalInput").ap()
    bias_d = nc.dram_tensor("bias", [TO], F32, kind="ExternalInput").ap()
    out_d = nc.dram_tensor("out", [TT, TO], F32, kind="ExternalOutput").ap()

    x_v = xT.rearrange("(kt p) t -> p kt t", p=128)
    w_v = wT.rearrange("(kt p) o -> p kt o", p=128)

    with tile.TileContext(nc) as tc:
        with (
            tc.tile_pool(name="ld", bufs=3) as p_ld,      # 8KB dma staging units
            tc.tile_pool(name="ax", bufs=4) as p_ax,      # 8KB abs/mult scratch
            tc.tile_pool(name="th", bufs=1) as p_th,      # 4KB tree scratch
            tc.tile_pool(name="wq", bufs=1) as p_wq,      # resident fp8 weights
            tc.tile_pool(name="xq", bufs=7) as p_xq,      # bf16 token-group tiles
            tc.tile_pool(name="sml", bufs=2) as p_sml,
            tc.tile_pool(name="rws", bufs=2) as p_rws,    # 1/w_scale, pass1->pass2
            tc.tile_pool(name="osb", bufs=2) as p_osb,
            tc.tile_pool(name="const", bufs=1) as p_const,
            tc.tile_pool(name="ps_mm", bufs=5, space="PSUM") as ps_mm,
            tc.tile_pool(name="ps_tr", bufs=2, space="PSUM") as ps_tr,
        ):
            ones11 = p_const.tile([1, 1], F32)
            nc.vector.memset(ones11[:], 1.0)
            mag_col = p_const.tile([128, 1], F32)
            nc.vector.memset(mag_col[:], MAGIC)
            nmag_col = p_const.tile([128, 1], F32)
            nc.vector.memset(nmag_col[:], -MAGIC)
            xs_cols = p_const.tile([128, NG], F32)    # x_scale, t on partitions
            ws_epi = p_const.tile([128, TO], BF16)    # w_scale bcast rows (epilogue)
            bias_bc = p_const.tile([128, TO], BF16)   # bias bcast rows
            nc.gpsimd.dma_start(
                out=bias_bc[:],
                in_=bass.AP(
                    tensor=bias_d.tensor, offset=bias_d.offset,
                    ap=[[0, 128], [1, TO]],
                ),
            )

            wq_oc = [
                p_wq.tile([128, KT, OC], FP8, name=f"wq_{oc}") for oc in range(NOC)
            ]
            xq_tiles = {}

            # ---------- x group: 2 half-units, amax tree, quantize ----------
            def x_group(tg):
                ts = slice(tg * GT, (tg + 1) * GT)
                xgs, axs = [], []
                for h in range(2):
                    xg = p_ld.tile([128, KH, GT], F32, tag="ld")
                    nc.sync.dma_start(
                        out=xg[:], in_=x_v[:, h * KH : (h + 1) * KH, ts]
                    )
                    xgs.append(xg)
                for h in range(2):
                    ax = p_ax.tile([128, KH, GT], F32, tag="ax")
                    nc.scalar.activation(
                        out=ax[:], in_=xgs[h][:],
                        func=mybir.ActivationFunctionType.Abs,
                    )
                    axs.append(ax)
                for h in range(2):
                    ax = axs[h]
                    th = p_th.tile([128, 8, GT], F32, tag="th")
                    # contiguous max tree over kt: 16->8->4->2->1
                    nc.vector.tensor_tensor(
                        out=th[:, 0:8, :], in0=ax[:, 0:8, :], in1=ax[:, 8:16, :],
                        op=A.max,
                    )
                    nc.vector.tensor_tensor(
                        out=ax[:, 0:4, :], in0=th[:, 0:4, :], in1=th[:, 4:8, :],
                        op=A.max,
                    )
                    nc.vector.tensor_tensor(
                        out=th[:, 0:2, :], in0=ax[:, 0:2, :], in1=ax[:, 2:4, :],
                        op=A.max,
                    )
                    nc.vector.tensor_tensor(
                        out=ax[:, 0:1, :], in0=th[:, 0:1, :], in1=th[:, 1:2, :],
                        op=A.max,
                    )
                am = p_sml.tile([128, GT], F32, tag="am")
                nc.vector.tensor_tensor(
                    out=am[:], in0=axs[0][:, 0, :], in1=axs[1][:, 0, :], op=A.max
                )
                am_bc = p_sml.tile([128, GT], F32, tag="ambc")
                nc.gpsimd.partition_all_reduce(
                    am_bc[:], am[:], 128, bass_isa.ReduceOp.absmax
                )
                xs_bc = p_sml.tile([128, GT], F32, tag="xsbc")
                nc.vector.tensor_scalar(
                    out=xs_bc[:], in0=am_bc[:], scalar1=1.0 / 127.0, scalar2=EPS,
                    op0=A.mult, op1=A.max,
                )
                rxs_bc = p_sml.tile([128, GT], F32, tag="rxsbc")
                nc.vector.reciprocal(rxs_bc[:], xs_bc[:])
                ptr = ps_tr.tile([128, 1], F32, tag="tr")
                nc.tensor.transpose(ptr[:], xs_bc[0:1, :], ones11[0:1, 0:1])
                nc.scalar.copy(xs_cols[:, tg : tg + 1], ptr[:])
                rxs_kt = bass.AP(
                    tensor=rxs_bc.tensor, offset=rxs_bc.offset,
                    ap=[rxs_bc.ap[0], [0, KH], rxs_bc.ap[1]],
                )
                xq_t = p_xq.tile([128, KT, GT], BF16, tag="xq")
                for h in range(2):
                    # xn = x * (1/xs) into the abs tile (free after the tree)
                    nc.gpsimd.tensor_tensor(
                        out=axs[h][:], in0=xgs[h][:], in1=rxs_kt, op=A.mult
                    )
                for h in range(2):
                    nc.vector.tensor_scalar(
                        out=xq_t[:, h * KH : (h + 1) * KH, :], in0=axs[h][:],
                        scalar1=MAGIC, scalar2=MAGIC, op0=A.add, op1=A.subtract,
                    )
                xq_tiles[tg] = xq_t

            # ---------- W pass 1: stream block, |w| row sums -> scales ----------
            def w_scales(ob):
                osl = slice(ob * OB, (ob + 1) * OB)
                run = None
                for qx in range(4):
                    wgf = p_ld.tile([128, KQ, OB], F32, tag="ld")
                    nc.sync.dma_start(
                        out=wgf[:], in_=w_v[:, qx * KQ : (qx + 1) * KQ, osl]
                    )
                    aw = p_ax.tile([128, KQ, OB], F32, tag="ax")
                    nc.scalar.activation(
                        out=aw[:], in_=wgf[:],
                        func=mybir.ActivationFunctionType.Abs,
                    )
                    th = p_th.tile([128, 4, OB], F32, tag="th")
                    # contiguous add tree over kt: 8->4->2->1 (partial in aw[2])
                    nc.vector.tensor_tensor(
                        out=th[:, 0:4, :], in0=aw[:, 0:4, :], in1=aw[:, 4:8, :],
                        op=A.add,
                    )
                    nc.vector.tensor_tensor(
                        out=aw[:, 0:2, :], in0=th[:, 0:2, :], in1=th[:, 2:4, :],
                        op=A.add,
                    )
                    nc.vector.tensor_tensor(
                        out=aw[:, 2:3, :], in0=aw[:, 0:1, :], in1=aw[:, 1:2, :],
                        op=A.add,
                    )
                    partial = aw[:, 2, :]
                    if run is None:
                        run = partial
                    else:
                        nrun = p_sml.tile([128, OB], F32, tag="run")
                        nc.vector.tensor_tensor(
                            out=nrun[:], in0=run, in1=partial, op=A.add
                        )
                        run = nrun[:]
                wsum_bc = p_sml.tile([128, OB], F32, tag="wsumbc")
                nc.gpsimd.partition_all_reduce(
                    wsum_bc[:], run, 128, bass_isa.ReduceOp.add
                )
                wsf = p_sml.tile([128, OB], F32, tag="wsf")
                nc.vector.tensor_scalar(
                    out=wsf[:], in0=wsum_bc[:],
                    scalar1=1.0 / K, scalar2=EPS, op0=A.mult, op1=A.max,
                )
                nc.vector.tensor_copy(ws_epi[:, osl], wsf[:])
                rws_bc = p_rws.tile([128, OB], F32, tag="rws")
                nc.vector.reciprocal(rws_bc[:], wsf[:])
                return rws_bc

            # ---------- W pass 2: re-stream block, quantize -> fp8 ----------
            def w_quant(ob, rws_bc):
                osl = slice(ob * OB, (ob + 1) * OB)
                rws_kt = bass.AP(
                    tensor=rws_bc.tensor, offset=rws_bc.offset,
                    ap=[rws_bc.ap[0], [0, KQ], rws_bc.ap[1]],
                )
                oc, oco = ob // 2, (ob % 2) * OB
                for qx in range(4):
                    wgf = p_ld.tile([128, KQ, OB], F32, tag="ld")
                    nc.sync.dma_start(
                        out=wgf[:], in_=w_v[:, qx * KQ : (qx + 1) * KQ, osl]
                    )
                    mq = p_ax.tile([128, KQ, OB], F32, tag="ax")
                    nc.gpsimd.tensor_tensor(
                        out=mq[:], in0=wgf[:], in1=rws_kt, op=A.mult
                    )
                    # v + M on the scalar engine: exact rint in the f32 add
                    nc.scalar.activation(
                        out=wgf[:], in_=mq[:],
                        func=mybir.ActivationFunctionType.Identity,
                        bias=mag_col[:],
                    )
                    # clip in magic domain: (u min M+1) max M-1
                    nc.vector.tensor_scalar(
                        out=mq[:], in0=wgf[:], scalar1=MAGIC + 1.0,
                        scalar2=MAGIC - 1.0, op0=A.min, op1=A.max,
                    )
                    # u - M on the scalar engine, cast to fp8 {-1,0,1}
                    nc.scalar.activation(
                        out=wq_oc[oc][:, qx * KQ : (qx + 1) * KQ, oco : oco + OB],
                        in_=mq[:],
                        func=mybir.ActivationFunctionType.Identity,
                        bias=nmag_col[:],
                    )

            def w_block_pair(ob0):
                r0 = w_scales(ob0)
                r1 = w_scales(ob0 + 1)
                w_quant(ob0, r0)
                w_quant(ob0 + 1, r1)

            # ---------- matmul pass ----------
            def mm_pass(tg, oc):
                xq_t = xq_tiles[tg]
                pm = ps_mm.tile([128, OC], F32, tag="mm")
                for kt in range(KT):
                    nc.tensor.matmul(
                        pm[:],
                        xq_t[:, kt, :],
                        wq_oc[oc][:, kt, :],
                        start=(kt == 0),
                        stop=(kt == KT - 1),
                    )
                osb = p_osb.tile([128, OC], F32, tag="osb")
                nc.vector.scalar_tensor_tensor(
                    out=osb[:], in0=pm[:], scalar=xs_cols[:, tg : tg + 1],
                    in1=ws_epi[:, oc * OC : (oc + 1) * OC], op0=A.mult, op1=A.mult,
                )
                nc.gpsimd.tensor_tensor(
                    out=osb[:], in0=osb[:],
                    in1=bias_bc[:, oc * OC : (oc + 1) * OC], op=A.add,
                )
                nc.sync.dma_start(
                    out=out_d[tg * GT : (tg + 1) * GT, oc * OC : (oc + 1) * OC],
                    in_=osb[:],
                )

            # ---------- main schedule ----------
            # Fill: first token section + W blocks 0,1 (for oc 0).
            x_group(0)
            x_group(1)
            w_block_pair(0)
            x_group(2)
            x_group(3)
            wb_next = 2
            for sec in range(4):
                for oc in range(NOC):
                    for tg in range(sec * 4, sec * 4 + 4):
                        mm_pass(tg, oc)
                    # W pair for oc+1 issued one slot ahead of its readers
                    if sec == 0 and wb_next < NOB:
                        w_block_pair(wb_next)
                        wb_next += 2
                    # prefetch next section's token groups (after the mms so
                    # parked rounds never sit ahead of this slot's epilogues)
                    if sec < 3 and oc in (1, 2):
                        g = sec * 4 + 4 + (oc - 1) * 2
                        x_group(g)
                        x_group(g + 1)
    nc.compile()
    return nc


_NC_CACHE = {}
LAST_EXEC_NS = None


def _get_nc():
    if "full" not in _NC_CACHE:
        _NC_CACHE["full"] = build_nc()
    return _NC_CACHE["full"]


def _run(x, weight, bias, trace=False):
    global LAST_EXEC_NS
    x = np.asarray(x, dtype=np.float32)
    weight = np.asarray(weight, dtype=np.float32)
    bias = np.asarray(bias, dtype=np.float32)

    xT = np.ascontiguousarray(x.reshape(T_FULL, I).T)  # [I, T]
    wT = np.ascontiguousarray(weight.T)  # [I, O]

    TT = T_FULL // TSPLIT
    TO = O // OSPLIT
    in_maps = []
    for c in range(N_CORES):
        ti, oj = divmod(c, OSPLIT)
        in_maps.append(
            {
                "xT": np.ascontiguousarray(xT[:, ti * TT : (ti + 1) * TT]),
                "wT": np.ascontiguousarray(wT[:, oj * TO : (oj + 1) * TO]),
                "bias": np.ascontiguousarray(bias[oj * TO : (oj + 1) * TO]),
            }
        )

    nc = _get_nc()
    res = run_bass_kernel_spmd(
        nc, in_maps, core_ids=list(range(N_CORES)), trace=trace
    )
    LAST_EXEC_NS = res.exec_time_ns

    out = np.empty((T_FULL, O), dtype=np.float32)
    for c in range(N_CORES):
        ti, oj = divmod(c, OSPLIT)
        out[ti * TT : (ti + 1) * TT, oj * TO : (oj + 1) * TO] = res.results[c]["out"]
    return out.reshape(B, S, O)


def kernel(x, weight, bias):
    return _run(x, weight, bias, trace=False)


def kernel_traced(x, weight, bias):
    _run(x, weight, bias, trace=True)
    return LAST_EXEC_NS


# revision 21
# speedup vs baseline: 1.1375x; 1.0405x over previous
# BitLinear (eval path) Trainium2 kernel: ternary weight quant + int8 activation
# quant + dense matmul, tensor-parallel over 8 NeuronCores.
#
# Math (per reference):
#   w_scale[o] = max(mean_k |W[o,k]|, EPS)
#   w_quant    = clip(round(W / w_scale), -1, 1)            (ternary)
#   x_scale[t] = max(max_k |x[t,k]| / 127, EPS)
#   x_quant    = round(x / x_scale)                          (int8 range)
#   out[t,o]   = (sum_k x_quant[t,k] * w_quant[o,k]) * x_scale[t] * w_scale[o] + bias[o]
#
# Exactness: w_quant exact in fp8e4, x_quant (|v|<=127) exact in bf16,
# partials exact in fp32 PSUM (<= 127*4096 < 2^24).
#
# Sharding: 4 token groups x 2 out-feature groups = 8 cores. Matmuls are
# token-major (bf16 x_quant stationary, fp8 weights moving 512 wide).
# Weights stream from HBM TWICE (pass 1: |w| row sums -> scales; pass 2:
# quantize) so no large raw-W SBUF residency is needed. Activation amax uses
# scalar-engine |.| + a contiguous max tree on DVE (strided reduces cost
# 2 cyc/elem); all DVE quant ops write fresh tiles (in-place loses the 2x
# perf mode); scale multiplies run on GpSimd. Everything is interleaved with
# the matmul passes in token-section order so the PE never waits long.
import numpy as np

import concourse.bacc as bacc
import concourse.bass as bass
import concourse.bass_isa as bass_isa
import concourse.tile as tile
from concourse import mybir
from concourse.bass_utils import run_bass_kernel_spmd

F32 = mybir.dt.float32
BF16 = mybir.dt.bfloat16
FP8 = mybir.dt.float8e4

EPS = 1e-5
MAGIC = 12582912.0  # 1.5 * 2^23: (x + MAGIC) - MAGIC == rint(x) for |x| < 2^22

B, S, I, O = 4, 2048, 4096, 4096
T_FULL = B * S
TSPLIT, OSPLIT = 4, 2
N_CORES = TSPLIT * OSPLIT

A = mybir.AluOpType


def build_nc(K=I, TO=O // OSPLIT, TT=T_FULL // TSPLIT):
    """Per-core program: xT [K, TT], wT [K, TO], bias [TO] -> out [TT, TO]."""
    KT = K // 128      # 32 k subtiles
    KH = 16            # x half-unit kt size
    KQ = 8             # W quarter-unit kt size
    GT = 128           # tokens per group
    NG = TT // GT      # 16 token groups
    OC = 512           # moving width per matmul
    NOC = TO // OC     # 4 o-chunks
    OB = 256           # W quant block o-columns
    NOB = TO // OB     # 8 W blocks

    nc = bacc.Bacc("TRN2", target_bir_lowering=False, debug=False)
    xT = nc.dram_tensor("xT", [K, TT], F32, kind="ExternalInput").ap()
    wT = nc.dram_tensor("wT", [K, TO], F32, kind="ExternalInput").ap()
    bias_d = nc.dram_tensor("bias", [TO], F32, kind="ExternalInput").ap()
    out_d = nc.dram_tensor("out", [TT, TO], F32, kind="ExternalOutput").ap()

    x_v = xT.rearrange("(kt p) t -> p kt t", p=128)
    w_v = wT.rearrange("(kt p) o -> p kt o", p=128)

    with tile.TileContext(nc) as tc:
        with (
            tc.tile_pool(name="ld", bufs=4) as p_ld,      # 16KB dma staging units
            tc.tile_pool(name="wq", bufs=1) as p_wq,      # resident fp8 weights
            tc.tile_pool(name="xq", bufs=6) as p_xq,      # bf16 token-group tiles
            tc.tile_pool(name="sml", bufs=2) as p_sml,
            tc.tile_pool(name="rws", bufs=2) as p_rws,    # 1/w_scale, pass1->pass2
            tc.tile_pool(name="osb", bufs=2) as p_osb,
            tc.tile_pool(name="const", bufs=1) as p_const,
            tc.tile_pool(name="ps_mm", bufs=5, space="PSUM") as ps_mm,
            tc.tile_pool(name="ps_tr", bufs=2, space="PSUM") as ps_tr,
        ):
            ones11 = p_const.tile([1, 1], F32)
            nc.vector.memset(ones11[:], 1.0)
            mag_col = p_const.tile([128, 1], F32)
            nc.vector.memset(mag_col[:], MAGIC)
            nmag_col = p_const.tile([128, 1], F32)
            nc.vector.memset(nmag_col[:], -MAGIC)
            nmag1_col = p_const.tile([128, 1], F32)
            nc.vector.memset(nmag1_col[:], -(MAGIC - 1.0))
            two_col = p_const.tile([128, 1], F32)
            nc.vector.memset(two_col[:], 2.0)
            one_col = p_const.tile([128, 1], F32)
            nc.vector.memset(one_col[:], 1.0)
            xs_cols = p_const.tile([128, NG], F32)    # x_scale, t on partitions
            ws_epi = p_const.tile([128, TO], BF16)    # w_scale bcast rows (epilogue)
            bias_bc = p_const.tile([128, TO], BF16)   # bias bcast rows
            nc.gpsimd.dma_start(
                out=bias_bc[:],
                in_=bass.AP(
                    tensor=bias_d.tensor, offset=bias_d.offset,
                    ap=[[0, 128], [1, TO]],
                ),
            )

            wq_oc = [
                p_wq.tile([128, KT, OC], FP8, name=f"wq_{oc}") for oc in range(NOC)
            ]
            xq_tiles = {}

            # ---------- x group: load 128 tokens, amax, quantize ----------
            def x_group(tg):
                ts = slice(tg * GT, (tg + 1) * GT)
                xg = p_ld.tile([128, KT, GT], F32, tag="ld")
                nc.sync.dma_start(out=xg[:], in_=x_v[:, :, ts])
                am = p_sml.tile([128, GT], F32, tag="am")
                nc.vector.tensor_reduce(
                    out=am[:],
                    in_=xg[:].rearrange("p kt t -> p t kt"),
                    axis=mybir.AxisListType.X,
                    op=A.max,
                    apply_absolute_value=True,
                )
                am_bc = p_sml.tile([128, GT], F32, tag="ambc")
                nc.gpsimd.partition_all_reduce(
                    am_bc[:], am[:], 128, bass_isa.ReduceOp.absmax
                )
                xs_bc = p_sml.tile([128, GT], F32, tag="xsbc")
                nc.vector.tensor_scalar(
                    out=xs_bc[:], in0=am_bc[:], scalar1=1.0 / 127.0, scalar2=EPS,
                    op0=A.mult, op1=A.max,
                )
                rxs_bc = p_sml.tile([128, GT], F32, tag="rxsbc")
                nc.vector.reciprocal(rxs_bc[:], xs_bc[:])
                ptr = ps_tr.tile([128, 1], F32, tag="tr")
                nc.tensor.transpose(ptr[:], xs_bc[0:1, :], ones11[0:1, 0:1])
                nc.scalar.copy(xs_cols[:, tg : tg + 1], ptr[:])
                rxs_kt = bass.AP(
                    tensor=rxs_bc.tensor, offset=rxs_bc.offset,
                    ap=[rxs_bc.ap[0], [0, KT], rxs_bc.ap[1]],
                )
                # x *= 1/xs in place on gpsimd
                nc.gpsimd.tensor_tensor(out=xg[:], in0=xg[:], in1=rxs_kt, op=A.mult)
                # round on the scalar engine: +M (f32 add rounds), then -M -> bf16
                nc.scalar.activation(
                    out=xg[:], in_=xg[:],
                    func=mybir.ActivationFunctionType.Identity, bias=mag_col[:],
                )
                xq_t = p_xq.tile([128, KT, GT], BF16, tag="xq")
                nc.scalar.activation(
                    out=xq_t[:], in_=xg[:],
                    func=mybir.ActivationFunctionType.Identity, bias=nmag_col[:],
                )
                xq_tiles[tg] = xq_t

            # ---------- W pass 1: stream block, |w| row sums -> scales ----------
            def w_scales(ob):
                osl = slice(ob * OB, (ob + 1) * OB)
                parts = []
                for h in range(2):
                    wgf = p_ld.tile([128, KH, OB], F32, tag="ld")
                    nc.sync.dma_start(
                        out=wgf[:], in_=w_v[:, h * KH : (h + 1) * KH, osl]
                    )
                    part = p_sml.tile([128, OB], F32, tag=f"part{h}")
                    nc.vector.tensor_reduce(
                        out=part[:],
                        in_=wgf[:].rearrange("p kt o -> p o kt"),
                        axis=mybir.AxisListType.X,
                        op=A.add,
                        apply_absolute_value=True,
                    )
                    parts.append(part)
                wsum = p_sml.tile([128, OB], F32, tag="run")
                nc.vector.tensor_tensor(
                    out=wsum[:], in0=parts[0][:], in1=parts[1][:], op=A.add
                )
                wsum_bc = p_sml.tile([128, OB], F32, tag="wsumbc")
                nc.gpsimd.partition_all_reduce(
                    wsum_bc[:], wsum[:], 128, bass_isa.ReduceOp.add
                )
                wsf = p_sml.tile([128, OB], F32, tag="wsf")
                nc.vector.tensor_scalar(
                    out=wsf[:], in0=wsum_bc[:],
                    scalar1=1.0 / K, scalar2=EPS, op0=A.mult, op1=A.max,
                )
                nc.vector.tensor_copy(ws_epi[:, osl], wsf[:])
                rws_bc = p_rws.tile([128, OB], F32, tag="rws")
                nc.vector.reciprocal(rws_bc[:], wsf[:])
                return rws_bc

            # ---------- W pass 2: re-stream block, quantize -> fp8 ----------
            # After u = v + M (exact rint in the f32 add), the ternary clip is
            # a Relu chain on the scalar engine (all values exact in f32):
            #   a = Relu(u - (M-1)) = max(r+1, 0)
            #   c = Relu(2 - a)     = 1 - clip(r, -1, 1)
            #   wq = 1 - c          (cast fp8)
            def w_quant(ob, rws_bc):
                osl = slice(ob * OB, (ob + 1) * OB)
                rws_kt = bass.AP(
                    tensor=rws_bc.tensor, offset=rws_bc.offset,
                    ap=[rws_bc.ap[0], [0, KH], rws_bc.ap[1]],
                )
                oc, oco = ob // 2, (ob % 2) * OB
                for h in range(2):
                    wgf = p_ld.tile([128, KH, OB], F32, tag="ld")
                    nc.sync.dma_start(
                        out=wgf[:], in_=w_v[:, h * KH : (h + 1) * KH, osl]
                    )
                    nc.gpsimd.tensor_tensor(
                        out=wgf[:], in0=wgf[:], in1=rws_kt, op=A.mult
                    )
                    nc.scalar.activation(
                        out=wgf[:], in_=wgf[:],
                        func=mybir.ActivationFunctionType.Identity,
                        bias=mag_col[:],
                    )
                    nc.scalar.activation(
                        out=wgf[:], in_=wgf[:],
                        func=mybir.ActivationFunctionType.Relu,
                        bias=nmag1_col[:],
                    )
                    nc.scalar.activation(
                        out=wgf[:], in_=wgf[:],
                        func=mybir.ActivationFunctionType.Relu,
                        scale=-1.0, bias=two_col[:],
                    )
                    nc.scalar.activation(
                        out=wq_oc[oc][:, h * KH : (h + 1) * KH, oco : oco + OB],
                        in_=wgf[:],
                        func=mybir.ActivationFunctionType.Identity,
                        scale=-1.0, bias=one_col[:],
                    )

            def w_block_pair(ob0):
                r0 = w_scales(ob0)
                r1 = w_scales(ob0 + 1)
                w_quant(ob0, r0)
                w_quant(ob0 + 1, r1)

            # ---------- matmul pass ----------
            def mm_pass(tg, oc):
                xq_t = xq_tiles[tg]
                pm = ps_mm.tile([128, OC], F32, tag="mm")
                for kt in range(KT):
                    nc.tensor.matmul(
                        pm[:],
                        xq_t[:, kt, :],
                        wq_oc[oc][:, kt, :],
                        start=(kt == 0),
                        stop=(kt == KT - 1),
                    )
                osb = p_osb.tile([128, OC], F32, tag="osb")
                nc.vector.scalar_tensor_tensor(
                    out=osb[:], in0=pm[:], scalar=xs_cols[:, tg : tg + 1],
                    in1=ws_epi[:, oc * OC : (oc + 1) * OC], op0=A.mult, op1=A.mult,
                )
                nc.gpsimd.tensor_tensor(
                    out=osb[:], in0=osb[:],
                    in1=bias_bc[:, oc * OC : (oc + 1) * OC], op=A.add,
                )
                nc.sync.dma_start(
                    out=out_d[tg * GT : (tg + 1) * GT, oc * OC : (oc + 1) * OC],
                    in_=osb[:],
                )

            # ---------- main schedule ----------
            # Fill: first token section + W blocks 0,1 (for oc 0).
            x_group(0)
            x_group(1)
            w_block_pair(0)
            x_group(2)
            x_group(3)
            wb_next = 2
            for sec in range(4):
                for oc in range(NOC):
                    for tg in range(sec * 4, sec * 4 + 4):
                        mm_pass(tg, oc)
                    # W pair for oc+1 issued one slot ahead of its readers
                    if sec == 0 and wb_next < NOB:
                        w_block_pair(wb_next)
                        wb_next += 2
                    # prefetch next section's token groups (after the mms so
                    # parked rounds never sit ahead of this slot's epilogues)
                    if sec < 3 and oc in (1, 2):
                        g = sec * 4 + 4 + (oc - 1) * 2
                        x_group(g)
                        x_group(g + 1)
    nc.compile()
    return nc


_NC_CACHE = {}
LAST_EXEC_NS = None


def _get_nc():
    if "full" not in _NC_CACHE:
        _NC_CACHE["full"] = build_nc()
    return _NC_CACHE["full"]


def _run(x, weight, bias, trace=False):
    global LAST_EXEC_NS
    x = np.asarray(x, dtype=np.float32)
    weight = np.asarray(weight, dtype=np.float32)
    bias = np.asarray(bias, dtype=np.float32)

    xT = np.ascontiguousarray(x.reshape(T_FULL, I).T)  # [I, T]
    wT = np.ascontiguousarray(weight.T)  # [I, O]

    TT = T_FULL // TSPLIT
    TO = O // OSPLIT
    in_maps = []
    for c in range(N_CORES):
        ti, oj = divmod(c, OSPLIT)
        in_maps.append(
            {
                "xT": np.ascontiguousarray(xT[:, ti * TT : (ti + 1) * TT]),
                "wT": np.ascontiguousarray(wT[:, oj * TO : (oj + 1) * TO]),
                "bias": np.ascontiguousarray(bias[oj * TO : (oj + 1) * TO]),
            }
        )

    nc = _get_nc()
    res = run_bass_kernel_spmd(
        nc, in_maps, core_ids=list(range(N_CORES)), trace=trace
    )
    LAST_EXEC_NS = res.exec_time_ns

    out = np.empty((T_FULL, O), dtype=np.float32)
    for c in range(N_CORES):
        ti, oj = divmod(c, OSPLIT)
        out[ti * TT : (ti + 1) * TT, oj * TO : (oj + 1) * TO] = res.results[c]["out"]
    return out.reshape(B, S, O)


def kernel(x, weight, bias):
    return _run(x, weight, bias, trace=False)


def kernel_traced(x, weight, bias):
    _run(x, weight, bias, trace=True)
    return LAST_EXEC_NS


# revision 22
# speedup vs baseline: 1.5966x; 1.4036x over previous
# BitLinear (eval path) Trainium2 kernel: ternary weight quant + int8 activation
# quant + dense matmul, tensor-parallel over 8 NeuronCores.
#
# Math (per reference):
#   w_scale[o] = max(mean_k |W[o,k]|, EPS)
#   w_quant    = clip(round(W / w_scale), -1, 1)            (ternary)
#   x_scale[t] = max(max_k |x[t,k]| / 127, EPS)
#   x_quant    = round(x / x_scale)                          (int8 range)
#   out[t,o]   = (sum_k x_quant[t,k] * w_quant[o,k]) * x_scale[t] * w_scale[o] + bias[o]
#
# Exactness: w_quant exact in fp8e4/bf16, x_quant (|v|<=127) exact in bf16,
# partials exact in fp32 PSUM (<= 127*4096 < 2^24). Magic-constant rounding
# (v + 1.5*2^23 rounds to rint(v) in the f32 add) and a Relu chain give the
# ternary clip entirely on the scalar engine.
#
# Layout strategy: both X and W are quantized in NATURAL row-major layout
# ([tokens, K] / [out_features, K]) where the per-row scales are
# per-PARTITION: the amax/abs-sum reduces are contiguous single DVE ops and
# the scale multiply + magic round fuse into one scalar-engine pass
# (func(in*scale + bias) with a [128,1] scale). The quantized bf16 rows are
# then transposed into the K-major matmul layout with the DMA xbar transpose
# (one dma_start_transpose per 128-row group). No GpSimd broadcasts, no
# partition reductions, W is read from HBM only once, and the host does no
# transposes at all. Matmuls are token-major: bf16 x_quant stationary, fp8
# weights moving 512 wide, fp32 PSUM, epilogue on DVE + GpSimd.
import numpy as np

import concourse.bacc as bacc
import concourse.bass as bass
import concourse.tile as tile
from concourse import mybir
from concourse.bass_utils import run_bass_kernel_spmd
from concourse.masks import make_identity

F32 = mybir.dt.float32
BF16 = mybir.dt.bfloat16
FP8 = mybir.dt.float8e4

EPS = 1e-5
MAGIC = 12582912.0  # 1.5 * 2^23

B, S, I, O = 4, 2048, 4096, 4096
T_FULL = B * S
TSPLIT, OSPLIT = 4, 2
N_CORES = TSPLIT * OSPLIT

A = mybir.AluOpType
AF = mybir.ActivationFunctionType


def build_nc(K=I, TO=O // OSPLIT, TT=T_FULL // TSPLIT):
    """Per-core program: x [TT, K], w [TO, K], bias [TO] -> out [TT, TO]."""
    KT = K // 128      # 32 k subtiles
    GT = 128           # tokens / out-rows per group
    NG = TT // GT      # 16 token groups
    NB = TO // GT      # 16 W blocks
    OC = 512           # moving width per matmul
    NOC = TO // OC     # 4 o-chunks

    nc = bacc.Bacc("TRN2", target_bir_lowering=False, debug=False)
    x_d = nc.dram_tensor("x", [TT, K], F32, kind="ExternalInput").ap()
    w_d = nc.dram_tensor("w", [TO, K], F32, kind="ExternalInput").ap()
    bias_d = nc.dram_tensor("bias", [TO], F32, kind="ExternalInput").ap()
    out_d = nc.dram_tensor("out", [TT, TO], F32, kind="ExternalOutput").ap()

    with tile.TileContext(nc) as tc:
        with (
            tc.tile_pool(name="ld", bufs=3) as p_ld,      # 16KB f32 row-major loads
            tc.tile_pool(name="bt", bufs=2) as p_bt,      # 8KB bf16 quantized rows
            tc.tile_pool(name="wst", bufs=2) as p_wst,    # 8KB transposed W staging
            tc.tile_pool(name="wq", bufs=1) as p_wq,      # resident fp8 weights
            tc.tile_pool(name="xq", bufs=5) as p_xq,      # bf16 K-major token tiles
            tc.tile_pool(name="sml", bufs=2) as p_sml,
            tc.tile_pool(name="osb", bufs=2) as p_osb,
            tc.tile_pool(name="const", bufs=1) as p_const,
            tc.tile_pool(name="ps_mm", bufs=5, space="PSUM") as ps_mm,
            tc.tile_pool(name="ps_tr", bufs=2, space="PSUM") as ps_tr,
        ):
            ident = p_const.tile([128, 128], F32)
            make_identity(nc, ident[:])
            mag_col = p_const.tile([128, 1], F32)
            nc.vector.memset(mag_col[:], MAGIC)
            nmag_col = p_const.tile([128, 1], F32)
            nc.vector.memset(nmag_col[:], -MAGIC)
            nmag1_col = p_const.tile([128, 1], F32)
            nc.vector.memset(nmag1_col[:], -(MAGIC - 1.0))
            two_col = p_const.tile([128, 1], F32)
            nc.vector.memset(two_col[:], 2.0)
            one_col = p_const.tile([128, 1], F32)
            nc.vector.memset(one_col[:], 1.0)
            xs_cols = p_const.tile([128, NG], F32)    # x_scale, t on partitions
            ws_epi = p_const.tile([128, TO], BF16)    # w_scale bcast rows
            bias_bc = p_const.tile([128, TO], BF16)   # bias bcast rows
            nc.gpsimd.dma_start(
                out=bias_bc[:],
                in_=bass.AP(
                    tensor=bias_d.tensor, offset=bias_d.offset,
                    ap=[[0, 128], [1, TO]],
                ),
            )

            wq_oc = [
                p_wq.tile([128, KT, OC], FP8, name=f"wq_{oc}") for oc in range(NOC)
            ]
            xq_tiles = {}

            # ---------- x group: rows in, amax, quantize, xbar transpose ----
            def x_group(tg):
                xg = p_ld.tile([128, K], F32, tag="ld")
                nc.sync.dma_start(out=xg[:], in_=x_d[tg * GT : (tg + 1) * GT, :])
                am = p_sml.tile([128, 1], F32, tag="am")
                nc.vector.tensor_reduce(
                    out=am[:], in_=xg[:], axis=mybir.AxisListType.X,
                    op=A.max, apply_absolute_value=True,
                )
                nc.vector.tensor_scalar(
                    out=xs_cols[:, tg : tg + 1], in0=am[:],
                    scalar1=1.0 / 127.0, scalar2=EPS, op0=A.mult, op1=A.max,
                )
                rxs = p_sml.tile([128, 1], F32, tag="rxs")
                nc.vector.reciprocal(rxs[:], xs_cols[:, tg : tg + 1])
                # u = x*(1/xs) + M : scale multiply and exact rint in one pass
                nc.scalar.activation(
                    out=xg[:], in_=xg[:], func=AF.Identity,
                    scale=rxs[:], bias=mag_col[:],
                )
                xot = p_bt.tile([128, K], BF16, tag="bt")
                nc.scalar.activation(
                    out=xot[:], in_=xg[:], func=AF.Identity, bias=nmag_col[:],
                )
                xq_t = p_xq.tile([128, KT, GT], BF16, tag="xq")
                nc.sync.dma_start_transpose(xq_t[:], xot[:])
                xq_tiles[tg] = xq_t

            # ---------- W block: rows in, scales, ternary, transpose --------
            def w_block(ob):
                wg = p_ld.tile([128, K], F32, tag="ld")
                nc.sync.dma_start(out=wg[:], in_=w_d[ob * GT : (ob + 1) * GT, :])
                wsum = p_sml.tile([128, 1], F32, tag="wsum")
                nc.vector.tensor_reduce(
                    out=wsum[:], in_=wg[:], axis=mybir.AxisListType.X,
                    op=A.add, apply_absolute_value=True,
                )
                wsf = p_sml.tile([128, 1], F32, tag="wsf")
                nc.vector.tensor_scalar(
                    out=wsf[:], in0=wsum[:], scalar1=1.0 / K, scalar2=EPS,
                    op0=A.mult, op1=A.max,
                )
                rws = p_sml.tile([128, 1], F32, tag="rws")
                nc.vector.reciprocal(rws[:], wsf[:])
                # ws column -> broadcast row slice of ws_epi (PE transpose)
                ptr = ps_tr.tile([1, 128], F32, tag="tr")
                nc.tensor.transpose(ptr[:], wsf[:], ident[:])
                wsrow = p_sml.tile([1, 128], BF16, tag="wsrow")
                nc.scalar.copy(wsrow[:], ptr[:])
                nc.gpsimd.partition_broadcast(
                    ws_epi[:, ob * GT : (ob + 1) * GT], wsrow[:]
                )
                # u = w*(1/ws) + M (exact rint); then the ternary Relu chain:
                # a = Relu(u-(M-1)) = max(r+1,0); c = Relu(2-a) = 1-clip(r)
                nc.scalar.activation(
                    out=wg[:], in_=wg[:], func=AF.Identity,
                    scale=rws[:], bias=mag_col[:],
                )
                nc.scalar.activation(
                    out=wg[:], in_=wg[:], func=AF.Relu, bias=nmag1_col[:],
                )
                wot = p_bt.tile([128, K], BF16, tag="bt")
                nc.scalar.activation(
                    out=wot[:], in_=wg[:], func=AF.Relu,
                    scale=-1.0, bias=two_col[:],
                )
                wstg = p_wst.tile([128, KT, GT], BF16, tag="wst")
                nc.sync.dma_start_transpose(wstg[:], wot[:])
                # wq = 1 - c, cast to fp8 {-1,0,1}, into the resident slice
                oc, osl = ob // (OC // GT), (ob % (OC // GT)) * GT
                nc.vector.tensor_scalar(
                    out=wq_oc[oc][:, :, osl : osl + GT], in0=wstg[:],
                    scalar1=-1.0, scalar2=1.0, op0=A.mult, op1=A.add,
                )

            # ---------- matmul pass ----------
            def mm_pass(tg, oc):
                xq_t = xq_tiles[tg]
                pm = ps_mm.tile([128, OC], F32, tag="mm")
                for kt in range(KT):
                    nc.tensor.matmul(
                        pm[:],
                        xq_t[:, kt, :],
                        wq_oc[oc][:, kt, :],
                        start=(kt == 0),
                        stop=(kt == KT - 1),
                    )
                osb = p_osb.tile([128, OC], F32, tag="osb")
                nc.vector.scalar_tensor_tensor(
                    out=osb[:], in0=pm[:], scalar=xs_cols[:, tg : tg + 1],
                    in1=ws_epi[:, oc * OC : (oc + 1) * OC], op0=A.mult, op1=A.mult,
                )
                nc.gpsimd.tensor_tensor(
                    out=osb[:], in0=osb[:],
                    in1=bias_bc[:, oc * OC : (oc + 1) * OC], op=A.add,
                )
                nc.sync.dma_start(
                    out=out_d[tg * GT : (tg + 1) * GT, oc * OC : (oc + 1) * OC],
                    in_=osb[:],
                )

            # ---------- main schedule ----------
            # Fill: first token section + W blocks 0-3 (o-chunk 0).
            x_group(0)
            x_group(1)
            w_block(0)
            w_block(1)
            x_group(2)
            w_block(2)
            w_block(3)
            x_group(3)
            wb_next = 4
            for sec in range(4):
                for oc in range(NOC):
                    for tg in range(sec * 4, sec * 4 + 4):
                        mm_pass(tg, oc)
                    # W blocks for oc+1 issued one slot ahead of their readers
                    if sec == 0 and wb_next < NB:
                        for _ in range(4):
                            w_block(wb_next)
                            wb_next += 1
                    # prefetch next section's token groups
                    if sec < 3 and oc in (1, 2):
                        g = sec * 4 + 4 + (oc - 1) * 2
                        x_group(g)
                        x_group(g + 1)
    nc.compile()
    return nc


_NC_CACHE = {}
LAST_EXEC_NS = None


def _get_nc():
    if "full" not in _NC_CACHE:
        _NC_CACHE["full"] = build_nc()
    return _NC_CACHE["full"]


def _run(x, weight, bias, trace=False):
    global LAST_EXEC_NS
    x = np.asarray(x, dtype=np.float32).reshape(T_FULL, I)
    weight = np.asarray(weight, dtype=np.float32)
    bias = np.asarray(bias, dtype=np.float32)

    TT = T_FULL // TSPLIT
    TO = O // OSPLIT
    in_maps = []
    for c in range(N_CORES):
        ti, oj = divmod(c, OSPLIT)
        in_maps.append(
            {
                "x": np.ascontiguousarray(x[ti * TT : (ti + 1) * TT, :]),
                "w": np.ascontiguousarray(weight[oj * TO : (oj + 1) * TO, :]),
                "bias": np.ascontiguousarray(bias[oj * TO : (oj + 1) * TO]),
            }
        )

    nc = _get_nc()
    res = run_bass_kernel_spmd(
        nc, in_maps, core_ids=list(range(N_CORES)), trace=trace
    )
    LAST_EXEC_NS = res.exec_time_ns

    out = np.empty((T_FULL, O), dtype=np.float32)
    for c in range(N_CORES):
        ti, oj = divmod(c, OSPLIT)
        out[ti * TT : (ti + 1) * TT, oj * TO : (oj + 1) * TO] = res.results[c]["out"]
    return out.reshape(B, S, O)


def kernel(x, weight, bias):
    return _run(x, weight, bias, trace=False)


def kernel_traced(x, weight, bias):
    _run(x, weight, bias, trace=True)
    return LAST_EXEC_NS
